# revision 11
# baseline (speedup 1.0000x reference)
"""Trainium2 Bass kernel for LocalCrossCorrelationWithSmoothnessLoss.

Full inputs in, full output out. Pure data-parallel over batch (B=8 -> 8
NeuronCores); each core computes partial sums for its image; the host
combines them into the three scalar losses.

Per-core pipeline (one 1024x1024 image pair + two flow channels):
  products  I16/J16 casts (GPSIMD), IJ (DVE), I2/J2 (ACT) -> f16 maps,
            10 row-chunks of <=128 rows (4-row conv halo baked in).
  pass 1    fused H-conv + transpose on the PE: stationary = data chunk
            [r_in, w 128], moving = banded box matrix [r_in, r' width]
            -> psum [w 128, r' 512-slice] f32.  Product maps use an
            81-scaled band.
  T-copy    psum -> SBUF f16 T maps [w 128, r' 1024] (DVE/ACT split).
  stage 2   W-conv: stationary = band [w 128, w' <=120], moving = T
            -> p2 psum [w', r' 512] f32 per map/half.
  combine   crossN = 81S_IJ - S_I*S_J, IvarN = 81S_II - S_I^2,
            JvarN = 81S_JJ - S_J^2, cc = (crossN * rsqrt(IvarN*JvarN))^2
            accumulated per-partition (DVE/ACT/GPSIMD split, bf16 temps).
  smooth    sum(s^2), lag_w = sum s[w]s[w+1] (DVE STT accum, fp32 2x);
            lag_h via PE shift-add matmul: t = s[p]+s[p+1] in psum, ACT
            Square-accum gives A = sum t^2; host recovers
            lag_h = (A - S - M)/2 from full (S) and partition-masked (M)
            column sums of s^2.  Tile-boundary rows fixed on the host.

Output per core: [2, 82] partial sums (row 1 = partition-0-masked).
Host assembles the losses in float64.
"""
import sys
import numpy as np

sys.path.insert(0, "/opt/trn_rl_repo")

import ml_dtypes
import bass_rust
import concourse.bass as bass
import concourse.tile as tile
from concourse import mybir
from concourse import bass_utils
from concourse import tile_utils

F32 = mybir.dt.float32
F32R = mybir.dt.float32r
F16 = mybir.dt.float16
BF16 = mybir.dt.bfloat16
ALU = mybir.AluOpType
ACTF = mybir.ActivationFunctionType

H = 1024
W = 1024
PAD = 4
WIN = 81.0
ALPHA = 0.01

# r'-chunks for pass-1 (out range, in range). 512-aligned slices:
# {120,120,120,120,32} x 2.  in = out +- PAD clamped to [0, H].
RCH = []
for _lo in (0, 120, 240, 360, 480, 512, 632, 752, 872, 992):
    _n = 32 if _lo in (480, 992) else 120
    _ilo = max(0, _lo - PAD)
    _ihi = min(H, _lo + _n + PAD)
    RCH.append((_lo, _n, _ilo, _ihi - _ilo))
NRC = len(RCH)

# w'-chunks for stage-2: out w' range + the 128-wide stationary col window.
WCH = []
for _j in range(9):
    _olo = 120 * _j
    _on = min(120, W - _olo)
    _clo = 0 if _j == 0 else (W - 128 if _olo + _on + PAD > W else _olo - PAD)
    WCH.append((_olo, _on, _clo))
NWC = len(WCH)

MAPS = ("si", "sj", "sij", "sii", "sjj")

# accumulator columns
ACC_CC = 0          # 18: (j, half)
ACC_S2 = 18         # 16: (ch, tile)
ACC_LW = 34         # 16
ACC_SH = 50         # 32: (ch, tile, half)
NACC = 82

tile_utils.max_sbuf_usage = 206 * 1024

_nc_cache = {}


def _legalize_waits(nc, max_waits=1):
    """walrus accepts only one sync-wait per instruction; split extras
    onto same-engine NoOps placed just before."""
    ctr = 0
    for f in nc.m.functions:
        for bb in f.blocks:
            insts = bb.instructions
            i = 0
            while i < len(insts):
                ins = insts[i]
                si = ins.sync_info
                if si is None:
                    i += 1
                    continue
                w = list(si.on_wait)
                if len(w) <= max_waits:
                    i += 1
                    continue
                extra, keep = w[:-max_waits], w[-max_waits:]
                nops = []
                for j in range(0, len(extra), max_waits):
                    chunk = extra[j:j + max_waits]
                    nop = mybir.InstNoOp(name=f"I-wsplit-{ctr}", ins=[], outs=[])
                    ctr += 1
                    nop.engine = ins.engine
                    nop.sync_info = bass_rust.SyncInfo(on_wait=chunk, on_update=[])
                    nops.append(nop)
                ins.sync_info = bass_rust.SyncInfo(on_wait=keep,
                                                  on_update=list(si.on_update))
                insts[i:i] = nops
                i += len(nops) + 1


def _act_raw(nc, out, in_, func, scale=1.0, accum_out=None):
    """InstActivation without the bass Rsqrt/Reciprocal guard."""
    se = nc.scalar
    bias = nc.const_aps.scalar_like(0.0, in_)
    ins = [se.lower_ap(in_), se.lower_ap(bias),
           mybir.ImmediateValue(dtype=mybir.dt.float32, value=scale),
           mybir.ImmediateValue(dtype=mybir.dt.float32, value=0.0)]
    outs = [se.lower_ap(out)]
    if accum_out is not None:
        outs.append(se.lower_ap(accum_out))
    return se.add_instruction(mybir.InstActivation(
        name=nc.get_next_instruction_name(), func=func, ins=ins, outs=outs))


def _band(klo, kn, olo, on, scale):
    k = np.arange(klo, klo + kn)[:, None]
    m = np.arange(olo, olo + on)[None, :]
    return (np.abs(k - m) <= PAD).astype(np.float32) * scale


def _make_host_consts():
    # bands tile [128, 544] f16:
    #   cols   0:120  B0    = |k - m|     <= 4   (unscaled)
    #   cols 120:240  Bmid  = |k - 4 - m| <= 4   (unscaled)
    #   cols 240:304  B8    = |k - 64 - m| <= 4  (unscaled, stage-2 j=8)
    #   cols 304:424  B0s   = B0 * 81
    #   cols 424:544  Bmids = Bmid * 81
    bands = np.zeros((128, 544), dtype=np.float32)
    bands[:, 0:120] = _band(0, 128, 0, 120, 1.0)
    bands[:, 120:240] = _band(0, 128, 4, 120, 1.0)
    bands[:, 240:304] = _band(0, 128, 64, 64, 1.0)
    bands[:, 304:424] = _band(0, 128, 0, 120, WIN)
    bands[:, 424:544] = _band(0, 128, 4, 120, WIN)
    bands_f16 = bands.astype(np.float16)

    # shift-add matrix [128, 128] f32: out[p] = s[p] + s[p+1]
    sadd = np.zeros((128, 128), dtype=np.float32)
    for p in range(128):
        sadd[p, p] = 1.0
        if p + 1 < 128:
            sadd[p + 1, p] = 1.0

    # ones [128, 2]: col 0 full, col 1 masks partition 0
    onesp = np.ones((128, 2), dtype=np.float32)
    onesp[0, 1] = 0.0
    return bands_f16, sadd, onesp


def _const_map(consts):
    bands_f16, sadd, onesp = consts
    return {"bands": bands_f16, "sadd": sadd, "onesp": onesp}


def _band_r(bands_t, c, scaled):
    """Moving band AP for pass-1 r-chunk c: [r_in rows, out cols]."""
    olo, on, ilo, inn = RCH[c]
    if c == 0:
        base = 304 if scaled else 0
    else:
        base = 424 if scaled else 120
    return bands_t[0:inn, base:base + on]


def _band_w(bands_t, j):
    """Stationary band AP for stage-2 w-chunk j: [128, out cols]."""
    olo, on, clo = WCH[j]
    if j == 0:
        return bands_t[0:128, 0:on]
    if olo - PAD == clo:
        return bands_t[0:128, 120:120 + on]
    return bands_t[0:128, 240:240 + on]


def _build(nc):
    I_d = nc.dram_tensor("I", [H, W], F32, kind="ExternalInput").ap()
    J_d = nc.dram_tensor("J", [H, W], F32, kind="ExternalInput").ap()
    s0_d = nc.dram_tensor("s0", [H, W], F32R, kind="ExternalInput").ap()
    s1_d = nc.dram_tensor("s1", [H, W], F32R, kind="ExternalInput").ap()
    bands_d = nc.dram_tensor("bands", [128, 544], F16,
                             kind="ExternalInput").ap()
    sadd_d = nc.dram_tensor("sadd", [128, 128], F32R,
                            kind="ExternalInput").ap()
    onesp_d = nc.dram_tensor("onesp", [128, 2], F32,
                             kind="ExternalInput").ap()
    part_d = nc.dram_tensor("partials", [2, NACC], F32,
                            kind="ExternalOutput").ap()

    from contextlib import ExitStack
    with tile.TileContext(nc) as tc, ExitStack() as ctx:
        consts = ctx.enter_context(tc.tile_pool(name="consts", bufs=1))
        inp = ctx.enter_context(tc.tile_pool(name="inp", bufs=2))
        xmap = ctx.enter_context(tc.tile_pool(name="xmap", bufs=1))
        tmap = ctx.enter_context(tc.tile_pool(name="tmap", bufs=2))
        ctmp = ctx.enter_context(tc.tile_pool(name="ctmp", bufs=2))
        spool = ctx.enter_context(tc.tile_pool(name="spool", bufs=3))
        sjunk = ctx.enter_context(tc.tile_pool(name="sjunk", bufs=2))
        accp = ctx.enter_context(tc.tile_pool(name="accp", bufs=1))
        psT = ctx.enter_context(tc.tile_pool(name="psT", bufs=3, space="PSUM"))
        ps2 = ctx.enter_context(tc.tile_pool(name="ps2", bufs=3, space="PSUM"))
        psS = ctx.enter_context(tc.tile_pool(name="psS", bufs=1, space="PSUM"))
        psF = ctx.enter_context(tc.tile_pool(name="psF", bufs=1, space="PSUM"))

        bands_t = consts.tile([128, 544], F16)
        sadd_t = consts.tile([128, 128], F32R)
        onesp_t = consts.tile([128, 2], F32)
        nc.scalar.dma_start(bands_t[:], bands_d)
        nc.scalar.dma_start(sadd_t[:], sadd_d)
        nc.scalar.dma_start(onesp_t[:], onesp_d)

        acc = accp.tile([128, NACC], F32)
        nc.vector.memset(acc[:], 0.0)

        # ---------------- emission helpers --------------------------------
        def load_rows(dst, src, r0, n, nslices, eng=None):
            """sliced HBM load on a HWDGE queue (sync or scalar)."""
            eng = eng or nc.sync
            step = (n + nslices - 1) // nslices
            o = 0
            while o < n:
                m = min(step, n - o)
                eng.dma_start(dst[o:o + m, :], src[r0 + o:r0 + o + m, :])
                o += m

        s_tiles_done = [0]

        def emit_s_tile():
            """one smoothness tile: load, s^2, lag_w, shift-add + A."""
            k = s_tiles_done[0]
            if k >= 16:
                return
            s_tiles_done[0] += 1
            ch, t = k // 8, k % 8
            s_d = s0_d if ch == 0 else s1_d
            st = spool.tile([128, W], F32R, tag="s_in")
            load_rows(st, s_d, 128 * t, 128, 2,
                      eng=(nc.sync if k % 2 == 0 else nc.scalar))
            # s^2 and lag_w on DVE (fp32 all-SBUF STT -> 2x mode)
            o1 = sjunk.tile([128, W], F32, tag="so1")
            nc.vector.scalar_tensor_tensor(
                out=o1[:], in0=st[:], scalar=1.0, in1=st[:],
                op0=ALU.mult, op1=ALU.mult,
                accum_out=acc[:, ACC_S2 + k:ACC_S2 + k + 1])
            o2 = sjunk.tile([128, W], F32, tag="so2")
            nc.vector.scalar_tensor_tensor(
                out=o2[:, 0:W - 1], in0=st[:, 1:W], scalar=1.0,
                in1=st[:, 0:W - 1], op0=ALU.mult, op1=ALU.mult,
                accum_out=acc[:, ACC_LW + k:ACC_LW + k + 1])
            # lag_h: t = s[p] + s[p+1] via PE, A = sum t^2 via ACT
            for hh in range(2):
                hsl = slice(512 * hh, 512 * hh + 512)
                pS = psS.tile([128, 512], F32, tag="psS")
                nc.tensor.matmul(pS[:, :], sadd_t[:], st[:, hsl],
                                 start=True, stop=True)
                o3 = sjunk.tile([128, 512], F32, tag="so3")
                col = ACC_SH + 2 * k + hh
                nc.scalar.activation(o3[:], pS[:, :], ACTF.Square,
                                     accum_out=acc[:, col:col + 1])

        # ---------------- products (emitted per r-slice) -------------------
        x_tiles = {}

        def emit_products(c):
            olo, on, ilo, inn = RCH[c]
            I_t = inp.tile([128, W], F32, tag="I_in")
            J_t = inp.tile([128, W], F32, tag="J_in")
            load_rows(I_t, I_d, ilo, inn, 3)
            load_rows(J_t, J_d, ilo, inn, 3, eng=nc.scalar)
            for name in MAPS:
                x_tiles[(name, c)] = xmap.tile([128, W], F16,
                                               tag=f"X_{name}_{c}",
                                               name=f"X_{name}_{c}")
            if c % 2 == 0:
                nc.gpsimd.tensor_copy(x_tiles[("si", c)][0:inn, :],
                                      I_t[0:inn, :])
                nc.gpsimd.tensor_copy(x_tiles[("sj", c)][0:inn, :],
                                      J_t[0:inn, :])
            else:
                nc.vector.tensor_copy(x_tiles[("si", c)][0:inn, :],
                                      I_t[0:inn, :])
                nc.scalar.copy(x_tiles[("sj", c)][0:inn, :], J_t[0:inn, :])
            nc.scalar.square(x_tiles[("sii", c)][0:inn, :], I_t[0:inn, :])
            nc.scalar.square(x_tiles[("sjj", c)][0:inn, :], J_t[0:inn, :])
            nc.vector.tensor_tensor(out=x_tiles[("sij", c)][0:inn, :],
                                    in0=I_t[0:inn, :], in1=J_t[0:inn, :],
                                    op=ALU.mult)

        for c in range(5):
            emit_products(c)
            if c % 2 == 1:
                emit_s_tile()

        # ------------- main loop: r-half outer, w-chunks inner -------------
        for hh in range(2):
            if hh == 1:
                # w-loop-A above only consumes chunks 0-4; load the rest now
                pass
            for j, (wolo, won, wclo) in enumerate(WCH):
                wsl = slice(wclo, wclo + 128)
                t_tiles = {}
                cs = list(range(5 * hh, 5 * hh + 5))
                for mi, name in enumerate(MAPS):
                    scaled = mi >= 2
                    t_tiles[name] = tmap.tile([128, 512], F16,
                                              tag=f"T_{name}",
                                              name=f"T_{name}_{j}_{hh}")
                    pT = psT.tile([128, 512], F32, tag="psT")
                    for c in cs:
                        olo, on, ilo, inn = RCH[c]
                        nc.tensor.matmul(
                            pT[0:128, olo - 512 * hh:olo - 512 * hh + on],
                            x_tiles[(name, c)][0:inn, wsl],
                            _band_r(bands_t, c, scaled),
                            start=(c == cs[0]), stop=(c == cs[-1]),
                            skip_group_check=True)
                    # T-copy psum -> SBUF f16, rotate DVE/ACT
                    if (j * 5 + mi) % 2 == 0:
                        nc.vector.tensor_copy(t_tiles[name][:, :], pT[:, :])
                    else:
                        nc.scalar.copy(t_tiles[name][:, :], pT[:, :])

                # stage 2: W-conv
                p2 = {}
                for name in MAPS:
                    p = ps2.tile([128, 512], F32, tag="p2")
                    nc.tensor.matmul(p[0:won, :], _band_w(bands_t, j),
                                     t_tiles[name][:, :],
                                     start=True, stop=True)
                    p2[name] = p

                # combine
                n = won
                si_p, sj_p = p2["si"], p2["sj"]
                sij_p, sii_p, sjj_p = p2["sij"], p2["sii"], p2["sjj"]
                si_sb = ctmp.tile([128, 512], BF16, tag="si_sb")
                nc.scalar.copy(si_sb[0:n, :], si_p[0:n, :])
                P = ctmp.tile([128, 512], BF16, tag="P")
                nc.vector.tensor_tensor(out=P[0:n, :], in0=si_sb[0:n, :],
                                        in1=sj_p[0:n, :], op=ALU.mult)
                crossN = ctmp.tile([128, 512], BF16, tag="crossN")
                nc.vector.tensor_tensor(out=crossN[0:n, :],
                                        in0=sij_p[0:n, :], in1=P[0:n, :],
                                        op=ALU.subtract)
                si2 = ctmp.tile([128, 512], BF16, tag="si2")
                nc.gpsimd.tensor_tensor(out=si2[0:n, :], in0=si_sb[0:n, :],
                                        in1=si_sb[0:n, :], op=ALU.mult)
                IvarN = ctmp.tile([128, 512], BF16, tag="IvarN")
                nc.vector.tensor_tensor(out=IvarN[0:n, :], in0=sii_p[0:n, :],
                                        in1=si2[0:n, :], op=ALU.subtract)
                sj2 = ctmp.tile([128, 512], BF16, tag="sj2")
                nc.scalar.square(sj2[0:n, :], sj_p[0:n, :])
                JvarN = ctmp.tile([128, 512], BF16, tag="JvarN")
                nc.vector.tensor_tensor(out=JvarN[0:n, :], in0=sjj_p[0:n, :],
                                        in1=sj2[0:n, :], op=ALU.subtract)
                denom = ctmp.tile([128, 512], F32, tag="denom")
                nc.gpsimd.tensor_tensor(out=denom[0:n, :], in0=IvarN[0:n, :],
                                        in1=JvarN[0:n, :], op=ALU.mult)
                rs = ctmp.tile([128, 512], BF16, tag="rs")
                _act_raw(nc, rs[0:n, :], denom[0:n, :], ACTF.Rsqrt)
                q = ctmp.tile([128, 512], BF16, tag="q")
                nc.gpsimd.tensor_tensor(out=q[0:n, :], in0=crossN[0:n, :],
                                        in1=rs[0:n, :], op=ALU.mult)
                qj = ctmp.tile([128, 512], BF16, tag="qj")
                col = ACC_CC + 2 * j + hh
                nc.vector.scalar_tensor_tensor(
                    out=qj[0:n, :], in0=q[0:n, :], scalar=1.0,
                    in1=q[0:n, :], op0=ALU.mult, op1=ALU.mult,
                    accum_out=acc[0:n, col:col + 1])

                if hh == 0 and j < 5:
                    # overlap w-loop-A compute with loading chunks 5-9
                    emit_products(5 + j)
                else:
                    emit_s_tile()

        while s_tiles_done[0] < 16:
            emit_s_tile()

        # ---------------- final partition reduction ------------------------
        pF = psF.tile([2, NACC], F32, tag="pF")
        nc.tensor.matmul(pF[:], onesp_t[:], acc[:], start=True, stop=True)
        outt = accp.tile([2, NACC], F32, tag="outt")
        nc.scalar.copy(outt[:], pF[:])
        nc.scalar.dma_start(part_d, outt[:])

    return


def _get_nc():
    if "nc" not in _nc_cache:
        nc = bass.Bass("TRN2", target_bir_lowering=False, debug=False)
        _build(nc)
        _legalize_waits(nc)
        _nc_cache["nc"] = nc
    return _nc_cache["nc"]


def kernel(I, J, s, sum_filt):
    B = I.shape[0]
    assert I.shape == (B, 1, H, W) and s.shape == (B, 2, H, W)
    nc = _get_nc()
    consts = _make_host_consts()

    in_maps = []
    for b in range(B):
        m = {
            "I": np.ascontiguousarray(I[b, 0]),
            "J": np.ascontiguousarray(J[b, 0]),
            "s0": np.ascontiguousarray(s[b, 0]),
            "s1": np.ascontiguousarray(s[b, 1]),
        }
        m.update(_const_map(consts))
        in_maps.append(m)
    res = bass_utils.run_bass_kernel_spmd(nc, in_maps,
                                          core_ids=list(range(B)))
    parts = np.stack([res.results[b]["partials"] for b in range(B)])
    parts = parts.astype(np.float64)  # [B, 2, NACC]

    s64 = s.astype(np.float64)
    cc_sum = float(parts[:, 0, ACC_CC:ACC_CC + 18].sum())
    s2_full = parts[:, 0, ACC_S2:ACC_S2 + 16]        # [B, 16]
    s2_mask = parts[:, 1, ACC_S2:ACC_S2 + 16]
    lag_w = parts[:, 0, ACC_LW:ACC_LW + 16].sum(axis=1)
    A = (parts[:, 0, ACC_SH::2] + parts[:, 0, ACC_SH + 1::2])  # [B, 16]
    lag_h = ((A - s2_full - s2_mask) / 2.0).sum(axis=1)
    s2 = s2_full.sum(axis=1)

    # tile-boundary lag_h pairs (rows 127/128, ...) per core
    rb = np.arange(127, H - 1, 128)
    lag_h = lag_h + (s64[:, :, rb, :] * s64[:, :, rb + 1, :]).sum(axis=(1, 2, 3))

    # edge corrections per core (both channels folded together)
    e_w = (s64[:, :, :, 0] ** 2).sum(axis=(1, 2)) + \
          (s64[:, :, :, -1] ** 2).sum(axis=(1, 2))
    e_h = (s64[:, :, 0, :] ** 2).sum(axis=(1, 2)) + \
          (s64[:, :, -1, :] ** 2).sum(axis=(1, 2))

    sum_dx2 = (2.0 * s2 - e_w - 2.0 * lag_w).sum()
    sum_dy2 = (2.0 * s2 - e_h - 2.0 * lag_h).sum()
    cnt = B * 2 * H * (W - 1)

    ncc_loss = -cc_sum / (B * H * W)
    smooth = 0.5 * (sum_dx2 / cnt + sum_dy2 / cnt) * ALPHA
    total = ncc_loss + smooth
    return np.array([total, ncc_loss, smooth], dtype=np.float32)
